# revision 25
# baseline (speedup 1.0000x reference)
"""MinGRU forward on 8 Trainium2 NeuronCores.

Reference computation (per batch b):
    k       = x @ Wz + bz                 # [T, H]
    z       = sigmoid(k)
    c       = 1 - z
    htilde  = g(x @ Wh + bh)              # g(a) = a+0.5 if a>=0 else sigmoid(a)
                                          #      = max(a+0.5, sigmoid(a))
    h[0]    = g(h_0)
    h[t]    = c[t-1]*h[t-1] + z[t-1]*htilde[t-1]   (t = 1..T)
    out     = h                           # [T+1, H]

The log-space cumlogsumexp in the reference is exactly this linear
recurrence (all quantities positive, coefficients in (0,1), so the
linear form is numerically stable; computing c as 1-z in fp32 is too —
when the cancellation in 1-z is bad, c*h_prev is negligible in h).

Sharding: data-parallel over batch, one batch per core, weights
replicated.

Device layout: matmuls run with H on the PSUM partition dim and T on
the free dim — the layout tensor_tensor_scan needs to run the
recurrence along T at vector speed. x is transposed AND cast to fp16
on the host, so the device only does plain (fast, parallel) DMA loads
— no DMA-transpose, which would serialize against every other DMA via
Tile's deadlock guard. The gate-bias columns and the scan's initial
state g(h_0) are precomputed host-side and shipped as one contiguous
[128, 32] tensor (per-element strided bias loads cost ~1k descriptors
each and starve the SDMA engines at kernel start). The device writes
out[:, 1:] transposed ([H, T]) in fp16 (5e-4 rounding vs the 2e-2
tolerance); the host writes column 0 = g(h_0), widens, and
untransposes during the unshard.

Schedule notes (from perfetto trace analysis of previous versions):
  - ~32 junk matmuls at kernel start warm the PE HAM clock gate
    (1.2 -> 2.4 GHz) while the first weight/x DMAs are in flight, and
    bridge into the real matmul stream so no >3.4us idle window ever
    re-throttles it.
  - The head is HBM-delivery-bound: 5 MB (weights + x chunk 0) at
    ~350 GB/s. The two HWDGE rings are ~176 GB/s FIFO pipes, so the
    head DMAs are split across both rings in exact consumption order,
    ~2.5 MB each, and x chunks 1/2 are issued later, ring-gated
    behind chunk 0's gate stores so they can't steal bandwidth.
  - Chunk 0 consumes weight k-slices in DMA arrival order: k-outer
    over m 0-3 (the full PSUM), then m-outer for m 4-7.
  - Engine split per m-tile: ACT does s,z (two sigmoids); DVE does
    g (stt), c = 1-z (tensor_scalar, 2x mode), and the scan; gpsimd
    does v = z*g. At 256-wide chunks ACT with three sigmoids plus
    dispatch gaps (~1.9us/m) would be slower than the PE (1.73us/m).
  - The last two chunks are 256 wide to shrink the post-matmul
    gate->scan->store tail.
"""

import numpy as np

B, T, D, H = 8, 4096, 1024, 1024
P = 128
TCH = 512                 # main time-chunk (one PSUM bank of fp32)
KO = D // P               # contraction tiles
MO = H // P               # output-channel tiles
CHUNKS = [512] * 7 + [256, 256]   # sum = 4096
NWARM = 32                # HAM warm-up matmuls
GQ = 4                    # chunk-0 pass-A group width (m 0..GQ-1)
NCONST = 5                # bz, -bz, bh, bh+0.5, g(h0) columns

_PROGRAM_CACHE = {}


def _build_program():
    import concourse.bacc as bacc
    import concourse.mybir as mybir
    import concourse.tile as tile

    fp32 = mybir.dt.float32
    fp16 = mybir.dt.float16
    SIG = mybir.ActivationFunctionType.Sigmoid
    MUL = mybir.AluOpType.mult
    ADD = mybir.AluOpType.add
    MAX = mybir.AluOpType.max
    SUB = mybir.AluOpType.subtract

    nc = bacc.Bacc("TRN2", target_bir_lowering=False)

    # x arrives pre-transposed from the host: [D, T]
    xt_ext = nc.declare_dram_parameter("x", [D, T], fp16, isOutput=False)
    wz_ext = nc.declare_dram_parameter("Wz", [D, H], fp16, isOutput=False)
    wh_ext = nc.declare_dram_parameter("Wh", [D, H], fp16, isOutput=False)
    # host-packed [P, 5*MO]: bz | -bz | bh | bh+0.5 | g(h0), each [P, MO]
    cn_ext = nc.declare_dram_parameter(
        "consts", [P, NCONST * MO], fp32, isOutput=False
    )
    # transposed fp16 output for t = 1..T; host adds the t=0 column and
    # widens to fp32
    out_ext = nc.declare_dram_parameter("out", [H, T], fp16, isOutput=True)

    xt_r = xt_ext.rearrange("(ko ki) t -> ki ko t", ki=P)
    wz_r = wz_ext.rearrange("(ko ki) h -> ki ko h", ki=P)
    wh_r = wh_ext.rearrange("(ko ki) h -> ki ko h", ki=P)

    chunk_t0 = []
    t0 = 0
    for w in CHUNKS:
        chunk_t0.append(t0)
        t0 += w
    NCH = len(CHUNKS)

    with tile.TileContext(nc) as tc:
        with (
            tc.tile_pool(name="const", bufs=1) as const_pool,
            tc.tile_pool(name="w", bufs=1) as w_pool,
            tc.tile_pool(name="xt", bufs=3) as xt_pool,
            tc.tile_pool(name="ht", bufs=2) as ht_pool,
            tc.tile_pool(name="gate", bufs=5) as gate_pool,
            tc.tile_pool(name="psp", bufs=4, space="PSUM") as psum_p,
        ):
            # --- PE warm-up: junk matmuls so the HAM clock gate is at
            # 2.4 GHz by the time the first real matmul's operands land.
            junk = const_pool.tile([P, P], fp16)
            nc.gpsimd.memset(junk, 0.0)
            warm_ps = psum_p.tile([P, P], fp32, tag="pk", name="warm")
            for _ in range(NWARM):
                nc.tensor.matmul(warm_ps, junk, junk, start=True, stop=True)

            wz_sb = w_pool.tile([P, KO, H], fp16)
            wh_sb = w_pool.tile([P, KO, H], fp16)
            cn_sb = const_pool.tile([P, NCONST * MO], fp32)
            bz_sb = cn_sb[:, 0 * MO:1 * MO]
            nbz_sb = cn_sb[:, 1 * MO:2 * MO]
            bh_sb = cn_sb[:, 2 * MO:3 * MO]
            bhp5_sb = cn_sb[:, 3 * MO:4 * MO]
            gh0_sb = cn_sb[:, 4 * MO:5 * MO]

            xt_tiles = {}

            def issue_xt(ci):
                tch = CHUNKS[ci]
                c0 = chunk_t0[ci]
                xt_sb = xt_pool.tile([P, KO, TCH], fp16, tag="xt", name="xt")
                nc.sync.dma_start(xt_sb[:, :, :tch], xt_r[:, :, c0:c0 + tch])
                xt_tiles[ci] = xt_sb

            # --- Head DMAs: chunk 0's x in per-ko slices, interleaved
            # with the weight k-slices across both rings in consumption
            # order (pk ko needs xt0[ko]+wz[ko], then pa ko needs
            # wh[ko]).
            xt0_sb = xt_pool.tile([P, KO, TCH], fp16, tag="xt", name="xt")
            xt_tiles[0] = xt0_sb

            def xt0k(ko):
                return (xt0_sb[:, ko:ko + 1], xt_r[:, ko:ko + 1, 0:TCH])

            # SP ring: wz + odd x0 slices + wh7
            nc.sync.dma_start(wz_sb[:, 0], wz_r[:, 0])
            for ko in range(1, KO):
                nc.sync.dma_start(*xt0k(ko)) if ko % 2 else None
                nc.sync.dma_start(wz_sb[:, ko], wz_r[:, ko])
            nc.sync.dma_start(wh_sb[:, KO - 1], wh_r[:, KO - 1])
            # ACT ring: even x0 slices + wh 0..6 + consts
            for i in range(4):
                nc.scalar.dma_start(*xt0k(2 * i))
                nc.scalar.dma_start(wh_sb[:, i], wh_r[:, i])
            nc.scalar.dma_start(cn_sb, cn_ext[:, :])
            for ko in range(4, KO - 1):
                nc.scalar.dma_start(wh_sb[:, ko], wh_r[:, ko])

            prev_ht = None  # previous chunk's scan output (carries the state)
            prev_tch = TCH
            pending = None  # software-pipelined (scan+store) of the prior m

            def scan_store(pd):
                # The scan runs one m-iteration behind the gates so that
                # on the DVE FIFO g(m) precedes scan(m-1): the gpsimd
                # v(m) then overlaps scan(m-1) instead of serializing
                # the chain scan -> g -> v -> scan (2.1 us/m, slower
                # than the PE's 1.73 us/m at 256-wide chunks).
                c_sb, v_sb, init, ht_sb, m, c0, tch = pd
                nc.vector.tensor_tensor_scan(
                    ht_sb[:, m, :tch], c_sb, v_sb, init, op0=MUL, op1=ADD
                )
                nc.sync.dma_start(
                    out_ext[m * P:(m + 1) * P, c0:c0 + tch],
                    ht_sb[:, m, :tch],
                )

            def gates_scan_store(m, c0, tch, pk, pa, ht_sb, light_dve=False):
                nonlocal pending
                # pa completes before pk (pa-first matmul order), so s/g
                # overlap the pk matmuls; z -> c/v -> scan is the tail
                # chain after pk.
                s_sb = gate_pool.tile([P, TCH], fp32, tag="s", name="s")[:, :tch]
                nc.scalar.activation(s_sb, pa, SIG, bias=bh_sb[:, m:m + 1])
                z_sb = gate_pool.tile([P, TCH], fp32, tag="z", name="z")[:, :tch]
                nc.scalar.activation(z_sb, pk, SIG, bias=bz_sb[:, m:m + 1])
                g_sb = gate_pool.tile([P, TCH], fp32, tag="g", name="g")[:, :tch]
                nc.vector.scalar_tensor_tensor(
                    g_sb, pa, bhp5_sb[:, m:m + 1], s_sb, op0=ADD, op1=MAX
                )
                if pending is not None:
                    scan_store(pending)
                c_sb = gate_pool.tile([P, TCH], fp32, tag="c", name="c")[:, :tch]
                v_sb = gate_pool.tile([P, TCH], fp32, tag="v", name="v")[:, :tch]
                # toward the kernel end the DVE must drain the wide
                # chunks' scan backlog faster than the narrow chunks'
                # PE cadence — alternate c between DVE (1-z) and a third
                # ACT sigmoid so neither engine exceeds the PE rate
                if light_dve and m % 2:
                    nc.scalar.activation(
                        c_sb, pk, SIG, bias=nbz_sb[:, m:m + 1], scale=-1.0
                    )
                else:
                    nc.vector.tensor_scalar(
                        c_sb, z_sb, 1.0, -1.0, op0=SUB, op1=MUL
                    )
                nc.gpsimd.tensor_mul(v_sb, z_sb, g_sb)

                init = (
                    gh0_sb[:, m:m + 1]
                    if prev_ht is None
                    else prev_ht[:, m, prev_tch - 1:prev_tch]
                )
                pending = (c_sb, v_sb, init, ht_sb, m, c0, tch)

            for ci in range(NCH):
                tch = CHUNKS[ci]
                c0 = chunk_t0[ci]
                if 1 <= ci < NCH - 2:
                    issue_xt(ci + 2)
                xt_sb = xt_tiles[ci]
                ht_sb = ht_pool.tile([P, MO, TCH], fp16)

                if ci == 0:
                    # Pass A (m 0..GQ-1): k-outer so matmuls consume the
                    # weight k-slices in DMA arrival order — the PE
                    # starts as soon as wz0 + x0[0] land and its 13.8 us
                    # of work spans most of the delivery window.
                    pks = [
                        psum_p.tile([P, TCH], fp32, tag="pk", name="pk")
                        for _ in range(GQ)
                    ]
                    pas = [
                        psum_p.tile([P, TCH], fp32, tag="pa", name="pa")
                        for _ in range(GQ)
                    ]
                    for ko in range(KO):
                        for q in range(GQ):
                            nc.tensor.matmul(
                                pks[q],
                                wz_sb[:, ko, q * P:(q + 1) * P],
                                xt_sb[:, ko, :tch],
                                start=(ko == 0),
                                stop=(ko == KO - 1),
                            )
                        for q in range(GQ):
                            nc.tensor.matmul(
                                pas[q],
                                wh_sb[:, ko, q * P:(q + 1) * P],
                                xt_sb[:, ko, :tch],
                                start=(ko == 0),
                                stop=(ko == KO - 1),
                            )
                    for q in range(GQ):
                        gates_scan_store(q, c0, tch, pks[q], pas[q], ht_sb)
                    # ring-gated: issues only once pass A's stores clear,
                    # so the 1 MB transfer doesn't steal SDMA bandwidth
                    # from the weight stream chunk 0 is consuming
                    issue_xt(1)
                    m_range = range(GQ, MO)
                else:
                    m_range = range(MO)

                for m in m_range:
                    pa = psum_p.tile([P, TCH], fp32, tag="pa", name="pa")[:, :tch]
                    pk = psum_p.tile([P, TCH], fp32, tag="pk", name="pk")[:, :tch]
                    for ko in range(KO):
                        nc.tensor.matmul(
                            pa,
                            wh_sb[:, ko, m * P:(m + 1) * P],
                            xt_sb[:, ko, :tch],
                            start=(ko == 0),
                            stop=(ko == KO - 1),
                        )
                    for ko in range(KO):
                        nc.tensor.matmul(
                            pk,
                            wz_sb[:, ko, m * P:(m + 1) * P],
                            xt_sb[:, ko, :tch],
                            start=(ko == 0),
                            stop=(ko == KO - 1),
                        )
                    gates_scan_store(
                        m, c0, tch, pk, pa, ht_sb, light_dve=(ci >= NCH - 3)
                    )

                if ci == 0:
                    issue_xt(2)
                prev_ht = ht_sb
                prev_tch = tch

            # flush the last m-tile's scan+store
            scan_store(pending)
            pending = None

    nc.finalize()
    return nc


def _get_program():
    if "v11" not in _PROGRAM_CACHE:
        _PROGRAM_CACHE["v11"] = _build_program()
    return _PROGRAM_CACHE["v11"]


def _g(x):
    return np.maximum(x + 0.5, 1.0 / (1.0 + np.exp(-x)))


def run(x, h_0, Wz, bz, Wh, bh, trace=False):
    from concourse.bass_utils import run_bass_kernel_spmd

    nc = _get_program()
    wz16 = np.ascontiguousarray(np.asarray(Wz, dtype=np.float16))
    wh16 = np.ascontiguousarray(np.asarray(Wh, dtype=np.float16))
    bz32 = np.asarray(bz, dtype=np.float32)
    bh32 = np.asarray(bh, dtype=np.float32)
    gh0 = _g(np.asarray(h_0, dtype=np.float32).reshape(B, H))  # [B, H]

    def col(v):  # [H] -> [P, MO] with partition = channel-within-tile
        return v.reshape(MO, P).T

    cn_common = [col(bz32), col(-bz32), col(bh32), col(bh32 + 0.5)]
    in_maps = [
        {
            "x": np.ascontiguousarray(np.asarray(x[b], dtype=np.float16).T),
            "Wz": wz16,
            "Wh": wh16,
            "consts": np.ascontiguousarray(
                np.concatenate(cn_common + [col(gh0[b])], axis=1),
                dtype=np.float32,
            ),
        }
        for b in range(B)
    ]
    res = run_bass_kernel_spmd(nc, in_maps, list(range(B)), trace=trace)
    out = np.empty((B, T + 1, H), dtype=np.float32)
    out[:, 0, :] = gh0
    for b in range(B):
        out[b, 1:, :] = res.results[b]["out"].T.astype(np.float32)
    return out, res


def kernel(x, h_0, Wz, bz, Wh, bh):
    out, _ = run(x, h_0, Wz, bz, Wh, bh)
    return out


# revision 26
# speedup vs baseline: 1.0631x; 1.0631x over previous
"""MinGRU forward on 8 Trainium2 NeuronCores.

Reference computation (per batch b):
    k       = x @ Wz + bz                 # [T, H]
    z       = sigmoid(k)
    c       = 1 - z
    htilde  = g(x @ Wh + bh)              # g(a) = a+0.5 if a>=0 else sigmoid(a)
                                          #      = max(a+0.5, sigmoid(a))
    h[0]    = g(h_0)
    h[t]    = c[t-1]*h[t-1] + z[t-1]*htilde[t-1]   (t = 1..T)
    out     = h                           # [T+1, H]

The log-space cumlogsumexp in the reference is exactly this linear
recurrence (all quantities positive, coefficients in (0,1), so the
linear form is numerically stable; computing c as 1-z in fp32 is too —
when the cancellation in 1-z is bad, c*h_prev is negligible in h).

Sharding: data-parallel over batch, one batch per core, weights
replicated.

Device layout: matmuls run with H on the PSUM partition dim and T on
the free dim — the layout tensor_tensor_scan needs to run the
recurrence along T at vector speed. x is transposed AND cast to fp16
on the host, so the device only does plain (fast, parallel) DMA loads
— no DMA-transpose, which would serialize against every other DMA via
Tile's deadlock guard. The gate-bias columns and the scan's initial
state g(h_0) are precomputed host-side and shipped as one contiguous
[128, 32] tensor (per-element strided bias loads cost ~1k descriptors
each and starve the SDMA engines at kernel start). The device writes
out[:, 1:] transposed ([H, T]) in fp16 (5e-4 rounding vs the 2e-2
tolerance); the host writes column 0 = g(h_0), widens, and
untransposes during the unshard.

Schedule notes (from perfetto trace analysis of previous versions):
  - ~32 junk matmuls at kernel start warm the PE HAM clock gate
    (1.2 -> 2.4 GHz) while the first weight/x DMAs are in flight, and
    bridge into the real matmul stream so no >3.4us idle window ever
    re-throttles it.
  - The head is HBM-delivery-bound: 5 MB (weights + x chunk 0) at
    ~350 GB/s. The two HWDGE rings are ~176 GB/s FIFO pipes, so the
    head DMAs are split across both rings in exact consumption order,
    ~2.5 MB each, and x chunks 1/2 are issued later, ring-gated
    behind chunk 0's gate stores so they can't steal bandwidth.
  - Chunk 0 consumes weight k-slices in DMA arrival order: k-outer
    over m 0-3 (the full PSUM), then m-outer for m 4-7.
  - Engine split per m-tile: ACT does s,z (two sigmoids); DVE does
    g (stt), c = 1-z (tensor_scalar, 2x mode), and the scan; gpsimd
    does v = z*g. At 256-wide chunks ACT with three sigmoids plus
    dispatch gaps (~1.9us/m) would be slower than the PE (1.73us/m).
  - The last two chunks are 256 wide to shrink the post-matmul
    gate->scan->store tail.
"""

import numpy as np

B, T, D, H = 8, 4096, 1024, 1024
P = 128
TCH = 512                 # main time-chunk (one PSUM bank of fp32)
KO = D // P               # contraction tiles
MO = H // P               # output-channel tiles
CHUNKS = [512] * 7 + [256, 256]   # sum = 4096
NWARM = 32                # HAM warm-up matmuls
GQ = 4                    # chunk-0 pass-A group width (m 0..GQ-1)
NCONST = 5                # bz, -bz, bh, bh+0.5, g(h0) columns

_PROGRAM_CACHE = {}


def _build_program():
    import concourse.bacc as bacc
    import concourse.mybir as mybir
    import concourse.tile as tile

    fp32 = mybir.dt.float32
    fp16 = mybir.dt.float16
    SIG = mybir.ActivationFunctionType.Sigmoid
    MUL = mybir.AluOpType.mult
    ADD = mybir.AluOpType.add
    MAX = mybir.AluOpType.max
    SUB = mybir.AluOpType.subtract

    nc = bacc.Bacc("TRN2", target_bir_lowering=False)

    # x arrives pre-transposed from the host: [D, T]
    xt_ext = nc.declare_dram_parameter("x", [D, T], fp16, isOutput=False)
    wz_ext = nc.declare_dram_parameter("Wz", [D, H], fp16, isOutput=False)
    wh_ext = nc.declare_dram_parameter("Wh", [D, H], fp16, isOutput=False)
    # host-packed [P, 5*MO]: bz | -bz | bh | bh+0.5 | g(h0), each [P, MO]
    cn_ext = nc.declare_dram_parameter(
        "consts", [P, NCONST * MO], fp32, isOutput=False
    )
    # transposed fp16 output for t = 1..T; host adds the t=0 column and
    # widens to fp32
    out_ext = nc.declare_dram_parameter("out", [H, T], fp16, isOutput=True)

    xt_r = xt_ext.rearrange("(ko ki) t -> ki ko t", ki=P)
    wz_r = wz_ext.rearrange("(ko ki) h -> ki ko h", ki=P)
    wh_r = wh_ext.rearrange("(ko ki) h -> ki ko h", ki=P)

    chunk_t0 = []
    t0 = 0
    for w in CHUNKS:
        chunk_t0.append(t0)
        t0 += w
    NCH = len(CHUNKS)

    with tile.TileContext(nc) as tc:
        with (
            tc.tile_pool(name="const", bufs=1) as const_pool,
            tc.tile_pool(name="w", bufs=1) as w_pool,
            tc.tile_pool(name="xt", bufs=3) as xt_pool,
            tc.tile_pool(name="ht", bufs=2) as ht_pool,
            tc.tile_pool(name="gate", bufs=5) as gate_pool,
            tc.tile_pool(name="psp", bufs=4, space="PSUM") as psum_p,
        ):
            # --- PE warm-up: junk matmuls so the HAM clock gate is at
            # 2.4 GHz by the time the first real matmul's operands land.
            junk = const_pool.tile([P, P], fp16)
            nc.gpsimd.memset(junk, 0.0)
            warm_ps = psum_p.tile([P, P], fp32, tag="pk", name="warm")
            for _ in range(NWARM):
                nc.tensor.matmul(warm_ps, junk, junk, start=True, stop=True)

            wz_sb = w_pool.tile([P, KO, H], fp16)
            wh_sb = w_pool.tile([P, KO, H], fp16)
            cn_sb = const_pool.tile([P, NCONST * MO], fp32)
            bz_sb = cn_sb[:, 0 * MO:1 * MO]
            nbz_sb = cn_sb[:, 1 * MO:2 * MO]
            bh_sb = cn_sb[:, 2 * MO:3 * MO]
            bhp5_sb = cn_sb[:, 3 * MO:4 * MO]
            gh0_sb = cn_sb[:, 4 * MO:5 * MO]

            xt_tiles = {}

            def issue_xt(ci):
                tch = CHUNKS[ci]
                c0 = chunk_t0[ci]
                xt_sb = xt_pool.tile([P, KO, TCH], fp16, tag="xt", name="xt")
                nc.sync.dma_start(xt_sb[:, :, :tch], xt_r[:, :, c0:c0 + tch])
                xt_tiles[ci] = xt_sb

            # --- Head DMAs: chunk 0's x in per-ko slices, interleaved
            # with the weight k-slices across both rings in consumption
            # order (pk ko needs xt0[ko]+wz[ko], then pa ko needs
            # wh[ko]).
            xt0_sb = xt_pool.tile([P, KO, TCH], fp16, tag="xt", name="xt")
            xt_tiles[0] = xt0_sb

            def xt0k(ko):
                return (xt0_sb[:, ko:ko + 1], xt_r[:, ko:ko + 1, 0:TCH])

            # SP ring: wz + odd x0 slices + wh7
            nc.sync.dma_start(wz_sb[:, 0], wz_r[:, 0])
            for ko in range(1, KO):
                nc.sync.dma_start(*xt0k(ko)) if ko % 2 else None
                nc.sync.dma_start(wz_sb[:, ko], wz_r[:, ko])
            nc.sync.dma_start(wh_sb[:, KO - 1], wh_r[:, KO - 1])
            # ACT ring: even x0 slices + wh 0..6 + consts
            for i in range(4):
                nc.scalar.dma_start(*xt0k(2 * i))
                nc.scalar.dma_start(wh_sb[:, i], wh_r[:, i])
            nc.scalar.dma_start(cn_sb, cn_ext[:, :])
            for ko in range(4, KO - 1):
                nc.scalar.dma_start(wh_sb[:, ko], wh_r[:, ko])

            prev_ht = None  # previous chunk's scan output (carries the state)
            prev_tch = TCH
            pending = None  # software-pipelined (scan+store) of the prior m

            def scan_store(pd):
                # The scan runs one m-iteration behind the gates so that
                # on the DVE FIFO g(m) precedes scan(m-1): the gpsimd
                # v(m) then overlaps scan(m-1) instead of serializing
                # the chain scan -> g -> v -> scan (2.1 us/m, slower
                # than the PE's 1.73 us/m at 256-wide chunks).
                c_sb, v_sb, init, ht_sb, m, c0, tch = pd
                nc.vector.tensor_tensor_scan(
                    ht_sb[:, m, :tch], c_sb, v_sb, init, op0=MUL, op1=ADD
                )
                nc.sync.dma_start(
                    out_ext[m * P:(m + 1) * P, c0:c0 + tch],
                    ht_sb[:, m, :tch],
                )

            def gates_scan_store(m, c0, tch, pk, pa, ht_sb, light_dve=False,
                                 last_chunk=False):
                nonlocal pending
                # pa completes before pk (pa-first matmul order), so s/g
                # overlap the pk matmuls; z -> c/v -> scan is the tail
                # chain after pk.
                s_sb = gate_pool.tile([P, TCH], fp32, tag="s", name="s")[:, :tch]
                nc.scalar.activation(s_sb, pa, SIG, bias=bh_sb[:, m:m + 1])
                z_sb = gate_pool.tile([P, TCH], fp32, tag="z", name="z")[:, :tch]
                nc.scalar.activation(z_sb, pk, SIG, bias=bz_sb[:, m:m + 1])
                g_sb = gate_pool.tile([P, TCH], fp32, tag="g", name="g")[:, :tch]
                nc.vector.scalar_tensor_tensor(
                    g_sb, pa, bhp5_sb[:, m:m + 1], s_sb, op0=ADD, op1=MAX
                )
                if pending is not None:
                    scan_store(pending)
                c_sb = gate_pool.tile([P, TCH], fp32, tag="c", name="c")[:, :tch]
                v_sb = gate_pool.tile([P, TCH], fp32, tag="v", name="v")[:, :tch]
                # toward the kernel end the DVE must drain the wide
                # chunks' scan backlog faster than the narrow chunks'
                # PE cadence — alternate c between DVE (1-z) and a third
                # ACT sigmoid so neither engine exceeds the PE rate
                if light_dve and m % 2:
                    nc.scalar.activation(
                        c_sb, pk, SIG, bias=nbz_sb[:, m:m + 1], scale=-1.0
                    )
                else:
                    nc.vector.tensor_scalar(
                        c_sb, z_sb, 1.0, -1.0, op0=SUB, op1=MUL
                    )
                # last chunk: v on DVE — at kernel drain the trailing
                # tiles otherwise ping-pong gpsimd(v) <-> DVE(scan) at
                # 2.1 us/tile with the PE already done
                if last_chunk:
                    nc.vector.tensor_mul(v_sb, z_sb, g_sb)
                else:
                    nc.gpsimd.tensor_mul(v_sb, z_sb, g_sb)

                init = (
                    gh0_sb[:, m:m + 1]
                    if prev_ht is None
                    else prev_ht[:, m, prev_tch - 1:prev_tch]
                )
                pending = (c_sb, v_sb, init, ht_sb, m, c0, tch)

            for ci in range(NCH):
                tch = CHUNKS[ci]
                c0 = chunk_t0[ci]
                if 1 <= ci < NCH - 2:
                    issue_xt(ci + 2)
                xt_sb = xt_tiles[ci]
                ht_sb = ht_pool.tile([P, MO, TCH], fp16)

                if ci == 0:
                    # Pass A (m 0..GQ-1): k-outer so matmuls consume the
                    # weight k-slices in DMA arrival order — the PE
                    # starts as soon as wz0 + x0[0] land and its 13.8 us
                    # of work spans most of the delivery window.
                    pks = [
                        psum_p.tile([P, TCH], fp32, tag="pk", name="pk")
                        for _ in range(GQ)
                    ]
                    pas = [
                        psum_p.tile([P, TCH], fp32, tag="pa", name="pa")
                        for _ in range(GQ)
                    ]
                    for ko in range(KO):
                        for q in range(GQ):
                            nc.tensor.matmul(
                                pks[q],
                                wz_sb[:, ko, q * P:(q + 1) * P],
                                xt_sb[:, ko, :tch],
                                start=(ko == 0),
                                stop=(ko == KO - 1),
                            )
                        for q in range(GQ):
                            nc.tensor.matmul(
                                pas[q],
                                wh_sb[:, ko, q * P:(q + 1) * P],
                                xt_sb[:, ko, :tch],
                                start=(ko == 0),
                                stop=(ko == KO - 1),
                            )
                    for q in range(GQ):
                        gates_scan_store(q, c0, tch, pks[q], pas[q], ht_sb)
                    # ring-gated: issues only once pass A's stores clear,
                    # so the 1 MB transfer doesn't steal SDMA bandwidth
                    # from the weight stream chunk 0 is consuming
                    issue_xt(1)
                    m_range = range(GQ, MO)
                else:
                    m_range = range(MO)

                for m in m_range:
                    pa = psum_p.tile([P, TCH], fp32, tag="pa", name="pa")[:, :tch]
                    pk = psum_p.tile([P, TCH], fp32, tag="pk", name="pk")[:, :tch]
                    for ko in range(KO):
                        nc.tensor.matmul(
                            pa,
                            wh_sb[:, ko, m * P:(m + 1) * P],
                            xt_sb[:, ko, :tch],
                            start=(ko == 0),
                            stop=(ko == KO - 1),
                        )
                    for ko in range(KO):
                        nc.tensor.matmul(
                            pk,
                            wz_sb[:, ko, m * P:(m + 1) * P],
                            xt_sb[:, ko, :tch],
                            start=(ko == 0),
                            stop=(ko == KO - 1),
                        )
                    gates_scan_store(
                        m, c0, tch, pk, pa, ht_sb,
                        light_dve=(ci >= NCH - 3),
                        last_chunk=(ci == NCH - 1),
                    )

                if ci == 0:
                    issue_xt(2)
                prev_ht = ht_sb
                prev_tch = tch

            # flush the last m-tile's scan+store
            scan_store(pending)
            pending = None

    nc.finalize()
    return nc


def _get_program():
    if "v12" not in _PROGRAM_CACHE:
        _PROGRAM_CACHE["v12"] = _build_program()
    return _PROGRAM_CACHE["v12"]


def _g(x):
    return np.maximum(x + 0.5, 1.0 / (1.0 + np.exp(-x)))


def run(x, h_0, Wz, bz, Wh, bh, trace=False):
    from concourse.bass_utils import run_bass_kernel_spmd

    nc = _get_program()
    wz16 = np.ascontiguousarray(np.asarray(Wz, dtype=np.float16))
    wh16 = np.ascontiguousarray(np.asarray(Wh, dtype=np.float16))
    bz32 = np.asarray(bz, dtype=np.float32)
    bh32 = np.asarray(bh, dtype=np.float32)
    gh0 = _g(np.asarray(h_0, dtype=np.float32).reshape(B, H))  # [B, H]

    def col(v):  # [H] -> [P, MO] with partition = channel-within-tile
        return v.reshape(MO, P).T

    cn_common = [col(bz32), col(-bz32), col(bh32), col(bh32 + 0.5)]
    in_maps = [
        {
            "x": np.ascontiguousarray(np.asarray(x[b], dtype=np.float16).T),
            "Wz": wz16,
            "Wh": wh16,
            "consts": np.ascontiguousarray(
                np.concatenate(cn_common + [col(gh0[b])], axis=1),
                dtype=np.float32,
            ),
        }
        for b in range(B)
    ]
    res = run_bass_kernel_spmd(nc, in_maps, list(range(B)), trace=trace)
    out = np.empty((B, T + 1, H), dtype=np.float32)
    out[:, 0, :] = gh0
    for b in range(B):
        out[b, 1:, :] = res.results[b]["out"].T.astype(np.float32)
    return out, res


def kernel(x, h_0, Wz, bz, Wh, bh):
    out, _ = run(x, h_0, Wz, bz, Wh, bh)
    return out
